# revision 8
# baseline (speedup 1.0000x reference)
"""CRF forward (alpha) recursion on 8 Trainium2 NeuronCores.

Strategy
--------
Data-parallel over batch: each core gets 32 of 256 batch rows.

Per core, the T=512 recurrence is run in *exp space*:
    A_{t+1}[nxt, b] = exp(x_{t+1}[nxt, b] - d) * sum_prev E[prev, nxt] * A_t[prev, b]
with E = exp(transition) loaded once as the PE stationary operand, and a
constant per-step normalizer d (the mean log-growth rate) keeping A in f32
range.  Each step is then exactly one small matmul (PE) + one elementwise
multiply (DVE).

The serial chain over T is broken via chunked speculation: T is split into
C=32 chunks of S=16 steps.  Every chunk runs K=8 warm-up steps on the
preceding chunk's data starting from a uniform state; the direction of the
CRF forward state forgets its initial condition at a measured rate of
<1e-7 in ~10 steps, so after warm-up each chunk's state equals the true
state up to a per-batch scalar.  The scalars are recovered exactly from
per-chunk column sums (ones-matmuls) and telescoped on the host in f64.
All 32 chunks advance in lockstep (24 slots instead of 512 serial steps),
packed 8 chunks per [128, 4*32] state tile (two 64x64 PE quadrants).

The masked transition column (into 'B', exactly -10000 => exp == 0 in f32)
makes alpha[:, 0] equal -inf in exp space; it is reconstructed exactly as
-10000 + lse(alpha_{T-1}) + x_{T-1, 0} from an extra ones-matmul.
"""

import numpy as np
from contextlib import ExitStack

import concourse.bass as bass
import concourse.bacc as bacc
import concourse.tile as tile
from concourse import mybir
from concourse.bass_utils import run_bass_kernel_spmd
from concourse.tile_rust import add_dep_helper

F32 = mybir.dt.float32
EXP = mybir.ActivationFunctionType.Exp

NCORES = 8
B, T, L = 256, 512, 64
BC = B // NCORES          # batch per core = 32
C = 32                    # chunks
S = T // C                # steps per chunk = 16
K = 8                     # warm-up steps
SL = K + S                # lockstep slots = 24
SG = 4                    # supergroups (8 chunks each: 4 in rows 0:64, 4 in rows 64:128)
NEG = -10000.0

# window-major X layout: windows 0,1 hold the "upper half" stripes (warm-up
# cells) of blocks 0..16, windows 2,3 the "lower half" stripes of blocks
# 1..16.  Widths in 128-col blocks:
WIN_BLKS = [17, 17, 16, 16]
WIN_BASE_BLK = [0, 17, 34, 50]       # cumulative, in blocks
NBLK = 66                            # total blocks
NCOL = NBLK * 128                    # 8448 f32 columns


def _mult_src(k, s):
    """(block_index_start, inner_col) of the Ex slice read by supergroup s
    at slot k: AP [128][4 blocks][32]."""
    if k < K:
        w = k // 4
        return WIN_BASE_BLK[w] + 4 * s, 32 * (k % 4)
    elif k < K + 8:
        w = 2 + (k - K) // 4
        return WIN_BASE_BLK[w] + 4 * s, 32 * ((k - K) % 4)
    else:
        w = (k - 16) // 4
        return WIN_BASE_BLK[w] + 4 * s + 1, 32 * ((k - 16) % 4)


def _build_program():
    nc = bacc.Bacc("TRN2", target_bir_lowering=False, debug=False,
                   num_devices=NCORES)
    xt_ap = nc.dram_tensor("xt", [128, NBLK, 128], F32, kind="ExternalInput").ap()
    bd_ap = nc.dram_tensor("bundle", [128, 98], F32, kind="ExternalInput").ap()
    af_ap = nc.dram_tensor("afinal", [64, 1, 32], F32, kind="ExternalOutput").ap()
    sm_ap = nc.dram_tensor("sums", [2, 1056], F32, kind="ExternalOutput").ap()

    with tile.TileContext(nc) as tc, ExitStack() as ctx:
        pc = ctx.enter_context(tc.tile_pool(name="const", bufs=1))
        px = ctx.enter_context(tc.tile_pool(name="x", bufs=1))
        pst = [ctx.enter_context(tc.tile_pool(name=f"st{s}", bufs=2))
               for s in range(SG)]
        # one PSUM bank per supergroup suffices: MM_{k+1} already waits on
        # mult_k (its rhs), which is also the WAR dependency for the bank
        pps = [ctx.enter_context(tc.tile_pool(name=f"ps{s}", bufs=1, space="PSUM"))
               for s in range(SG)]
        psm = ctx.enter_context(tc.tile_pool(name="psums", bufs=1, space="PSUM"))

        # ---- preamble ----
        bundle = pc.tile([128, 98], F32)
        nc.sync.dma_start(bundle[:], bd_ap)
        # bundle cols: 0:64 transition (stacked twice vertically), 64:96
        # onehot reset (rows 0:64), 96 ones, 97 -d bias

        # walrus in this pipeline encodes at most ONE semaphore wait per
        # compute/NoOp instruction, so the whole program is structured to a
        # strict 1-wait discipline: the only pre-barrier op is the bundle
        # DMA (barrier NOP waits on its queue sem alone), every post-barrier
        # instruction joins at most one foreign engine stream, and joins of
        # two streams go through same-engine observer ops.
        tc.strict_bb_all_engine_barrier()

        states = []
        for s in range(SG):
            t0 = pst[s].tile([128, 4, 32], F32, tag="st")
            nc.vector.memset(t0[:], 1.0)
            states.append(t0)
        collectA = pc.tile([1, 1056], F32)
        nc.vector.memset(collectA[:], 0.0)
        scratch = pc.tile([1, 4], F32)
        nc.vector.memset(scratch[:], 0.0)

        # E = exp(transition); reads bundle (covered by the barrier => no wait)
        E = pc.tile([128, 64], F32)
        nc.scalar.activation(E[:], bundle[:, 0:64], EXP, bias=0.0, scale=1.0)
        collectB = pc.tile([1, 1056], F32)
        nc.scalar.activation(collectB[:], collectA[:], mybir.ActivationFunctionType.Copy)

        # ---- X streaming: 4 big window DMAs (SP ring, FIFO order) ----
        xraw = px.tile([128, NBLK, 128], F32)
        ex = px.tile([128, NBLK, 128], F32)
        for w in range(4):
            b0, b1 = WIN_BASE_BLK[w], WIN_BASE_BLK[w] + WIN_BLKS[w]
            nc.sync.dma_start(xraw[:, b0:b1, :], xt_ap[:, b0:b1, :])
        bias_ap = bundle[:, 97:98]
        exp_last = {}
        for w in range(4):
            for g in range(4):
                b0 = WIN_BASE_BLK[w] + 4 * g
                b1 = WIN_BASE_BLK[w] + (4 * g + 4 if g < 3 else WIN_BLKS[w])
                inst = nc.scalar.activation(ex[:, b0:b1, :], xraw[:, b0:b1, :],
                                            EXP, bias=bias_ap, scale=1.0)
                exp_last[w] = inst

        # ---- main lockstep loop ----
        ones_lo = bundle[0:64, 96:97]
        ones_hi = bundle[64:128, 96:97]
        start_ps = psm.tile([128, 512], F32)
        end_ps = psm.tile([128, 512], F32)
        preT_ps = psm.tile([128, 32], F32)

        # junk matmul: PE observes the DVE memset tick so the first real
        # matmuls carry only the ACT (E) wait
        nc.tensor.matmul(start_ps[0:1, 0:128], lhsT=states[0][0:64, 0, 0:1],
                         rhs=states[0][0:64], start=True, stop=True,
                         tile_position=(0, 0))

        prev22_sg3 = None
        for k in range(SL):
            dummy = None
            if k in (0, 4, 8, 12):
                w = k // 4 if k < K else 2 + (k - K) // 4
                lb = WIN_BASE_BLK[w] + WIN_BLKS[w] - 1
                dummy = nc.vector.tensor_copy(scratch[0:1, 0:1],
                                              ex[0:1, lb, 0:1])
            new_states = []
            for s in range(SG):
                ps = pps[s].tile([128, 4, 32], F32, tag="ps")
                nc.tensor.matmul(ps[0:64], lhsT=E[0:64, :], rhs=states[s][0:64],
                                 start=True, stop=True, tile_position=(0, 0))
                nc.tensor.matmul(ps[64:128], lhsT=E[64:128, :],
                                 rhs=states[s][64:128],
                                 start=True, stop=True, tile_position=(64, 64))
                bix, u = _mult_src(k, s)
                nst = pst[s].tile([128, 4, 32], F32, tag="st")
                m = nc.vector.tensor_mul(nst[:], ps[:],
                                         ex[:, bix:bix + 4, u:u + 32])
                if dummy is not None:
                    add_dep_helper(m.ins, dummy.ins, sync=False,
                                   reason="observe new exp piece first")
                new_states.append(nst)
            states = new_states

            if k == K - 1:
                # reset chunk 0 (sg 0, rows 0:64, chunk col 0) to the exact
                # one-hot init, then record per-chunk start sums
                nc.vector.tensor_copy(states[0][0:64, 0, :],
                                      bundle[0:64, 64:96])
                for s in range(SG):
                    nc.tensor.matmul(start_ps[0:1, s * 128:(s + 1) * 128],
                                     lhsT=ones_lo, rhs=states[s][0:64],
                                     start=True, stop=True,
                                     tile_position=(0, 0))
                    nc.tensor.matmul(start_ps[64:65, s * 128:(s + 1) * 128],
                                     lhsT=ones_hi, rhs=states[s][64:128],
                                     start=True, stop=True,
                                     tile_position=(64, 64))
                nc.vector.tensor_copy(collectA[0:1, 0:512], start_ps[0:1, :])
                nc.scalar.copy(collectB[0:1, 0:512], start_ps[64:65, :])
            if k == SL - 2:
                prev22_sg3 = states[3]

        # preT: chunk 31 state before its last step
        nc.tensor.matmul(preT_ps[64:65, :], lhsT=ones_hi,
                         rhs=prev22_sg3[64:128, 3, :],
                         start=True, stop=True, tile_position=(64, 64))
        for s in range(SG):
            nc.tensor.matmul(end_ps[0:1, s * 128:(s + 1) * 128],
                             lhsT=ones_lo, rhs=states[s][0:64],
                             start=True, stop=True, tile_position=(0, 0))
            nc.tensor.matmul(end_ps[64:65, s * 128:(s + 1) * 128],
                             lhsT=ones_hi, rhs=states[s][64:128],
                             start=True, stop=True, tile_position=(64, 64))
        nc.vector.tensor_copy(collectA[0:1, 512:1024], end_ps[0:1, :])
        nc.scalar.copy(collectB[0:1, 512:1024], end_ps[64:65, :])
        nc.scalar.copy(collectB[0:1, 1024:1056], preT_ps[64:65, :])

        nc.sync.dma_start(af_ap, states[3][64:128, 3:4, :])
        nc.sync.dma_start(sm_ap[0:1, :], collectA[:])
        nc.sync.dma_start(sm_ap[1:2, :], collectB[:])
    nc.compile()
    return nc


_prog_cache = {}


def _get_program():
    if "nc" not in _prog_cache:
        _prog_cache["nc"] = _build_program()
    return _prog_cache["nc"]


def _compute_d(X, transition):
    """Mean per-step log growth of the total exp-space mass, from a short
    host-side probe.  Any value within ~+-0.1 keeps A in f32 range."""
    E = np.exp(transition.astype(np.float64))
    a = np.zeros((16, L), np.float64)
    a[:, 0] = 1.0
    tot, n = 0.0, 0
    for t in range(96):
        a = np.exp(X[:16, t, :].astype(np.float64)) * (a @ E)
        sm = a.sum()
        a /= sm
        if t >= 4:
            tot += np.log(sm)
            n += 1
    return float(np.clip(tot / n, 4.5, 5.9))


def _pack_core(Xc, d):
    """Xc [32, T, L] -> window-major device layout [128, NBLK, 128] f32."""
    Y = np.ascontiguousarray(Xc.transpose(2, 1, 0)).astype(np.float32)  # [L, T, 32]
    out = np.zeros((128, NBLK, 128), np.float32)
    # windows 0,1: upper-half stripes k in [4w, 4w+4) of blocks j=0..16
    for w in (0, 1):
        for j in range(17):
            for kk in range(4):
                k = 4 * w + kk
                t_loc = 16 * j - 8 + k
                dst = out[:, WIN_BASE_BLK[w] + j, 32 * kk:32 * kk + 32]
                if t_loc >= 0:
                    dst[0:64] = Y[:, t_loc, :]
                dst[64:128] = Y[:, 256 + t_loc, :]
    # windows 2,3: lower-half stripes m in [4(w-2), 4(w-2)+4) of blocks 1..16
    for w in (2, 3):
        for jj in range(16):
            for kk in range(4):
                m = 4 * (w - 2) + kk
                t_loc = 16 * jj + m
                dst = out[:, WIN_BASE_BLK[w] + jj, 32 * kk:32 * kk + 32]
                dst[0:64] = Y[:, t_loc, :]
                dst[64:128] = Y[:, 256 + t_loc, :]
    return out


def _make_bundle(transition, d):
    bd = np.zeros((128, 98), np.float32)
    tr = transition.astype(np.float32)
    bd[0:64, 0:64] = tr
    bd[64:128, 0:64] = tr
    bd[0, 64:96] = 1.0            # one-hot rows: row B_IDX=0 of the reset block
    bd[:, 96] = 1.0               # ones column
    bd[:, 97] = -d                # exp bias
    return bd


def kernel(X, transition):
    X = np.asarray(X, dtype=np.float32)
    transition = np.asarray(transition, dtype=np.float32)
    d = _compute_d(X, transition)

    bundle = _make_bundle(transition, d)
    in_maps = []
    for c in range(NCORES):
        xt = _pack_core(X[c * BC:(c + 1) * BC], d)
        in_maps.append({"xt": xt, "bundle": bundle})

    nc = _get_program()
    res = run_bass_kernel_spmd(nc, in_maps, core_ids=list(range(NCORES)))

    alpha = np.empty((B, L), np.float64)
    dS = float(d) * S
    with np.errstate(divide="ignore"):
        for c in range(NCORES):
            r = res.results[c]
            sums = r["sums"].astype(np.float64)
            af = r["afinal"].reshape(64, 32).astype(np.float64)
            start = np.empty((C, BC))
            end = np.empty((C, BC))
            start[:16] = sums[0, 0:512].reshape(16, BC)
            start[16:] = sums[1, 0:512].reshape(16, BC)
            end[:16] = sums[0, 512:1024].reshape(16, BC)
            end[16:] = sums[1, 512:1024].reshape(16, BC)
            preT = sums[1, 1024:1056]
            lam = np.zeros(BC)
            for cc in range(C - 1):
                lam += dS + np.log(end[cc]) - np.log(start[cc])
            base = lam - np.log(start[C - 1])
            blk = alpha[c * BC:(c + 1) * BC]
            blk[:] = (base[:, None] + dS + np.log(af).T)
            lse_preT = base + (dS - d) + np.log(preT)
            blk[:, 0] = NEG + lse_preT + X[c * BC:(c + 1) * BC, T - 1, 0].astype(np.float64)
    return alpha.astype(np.float32)


# revision 10
# speedup vs baseline: 1.4813x; 1.4813x over previous
"""CRF forward (alpha) recursion on 8 Trainium2 NeuronCores.

Strategy
--------
Data-parallel over batch: each core gets 32 of 256 batch rows.

Per core, the T=512 recurrence is run in *exp space*:
    A_{t+1}[nxt, b] = exp(x_{t+1}[nxt, b] - d) * sum_prev E[prev, nxt] * A_t[prev, b]
with E = exp(transition) loaded once as the PE stationary operand, and a
constant per-step normalizer d (the mean log-growth rate) keeping A in f32
range.  Each step is then exactly one small matmul (PE) + one elementwise
multiply (DVE).

The serial chain over T is broken via chunked speculation: T is split into
C=32 chunks of S=16 steps.  Every chunk runs K=8 warm-up steps on the
preceding chunk's data starting from a uniform state; the direction of the
CRF forward state forgets its initial condition at a measured rate of
<1e-7 in ~10 steps, so after warm-up each chunk's state equals the true
state up to a per-batch scalar.  The scalars are recovered exactly from
per-chunk column sums (ones-matmuls) and telescoped on the host in f64.
All 32 chunks advance in lockstep (24 slots instead of 512 serial steps),
packed 8 chunks per [128, 4*32] state tile (two 64x64 PE quadrants).

The masked transition column (into 'B', exactly -10000 => exp == 0 in f32)
makes alpha[:, 0] equal -inf in exp space; it is reconstructed exactly as
-10000 + lse(alpha_{T-1}) + x_{T-1, 0} from an extra ones-matmul.
"""

import numpy as np
from contextlib import ExitStack

import concourse.bass as bass
import concourse.bacc as bacc
import concourse.tile as tile
from concourse import mybir
from concourse.bass_utils import run_bass_kernel_spmd
from concourse.tile_rust import add_dep_helper

F32 = mybir.dt.float32
F32R = mybir.dt.float32r
EXP = mybir.ActivationFunctionType.Exp

NCORES = 8
B, T, L = 256, 512, 64
BC = B // NCORES          # batch per core = 32
C = 32                    # chunks
S = T // C                # steps per chunk = 16
K = 8                     # warm-up steps
SL = K + S                # lockstep slots = 24
SG = 2                    # supergroups (16 chunks each: 8 in rows 0:64, 8 in rows 64:128)
SGW = 8                   # chunks per supergroup row-half
NEG = -10000.0

# window-major X layout: windows 0,1 hold the "upper half" stripes (warm-up
# cells) of blocks 0..16, windows 2,3 the "lower half" stripes of blocks
# 1..16.  Widths in 128-col blocks:
WIN_BLKS = [17, 17, 16, 16]
WIN_BASE_BLK = [0, 17, 34, 50]       # cumulative, in blocks
NBLK = 66                            # total blocks
NCOL = NBLK * 128                    # 8448 f32 columns


def _mult_src(k, s):
    """(block_index_start, inner_col) of the Ex slice read by supergroup s
    at slot k: AP [128][SGW blocks][32]."""
    if k < K:
        w = k // 4
        return WIN_BASE_BLK[w] + SGW * s, 32 * (k % 4)
    elif k < K + 8:
        w = 2 + (k - K) // 4
        return WIN_BASE_BLK[w] + SGW * s, 32 * ((k - K) % 4)
    else:
        w = (k - 16) // 4
        return WIN_BASE_BLK[w] + SGW * s + 1, 32 * ((k - 16) % 4)


def _build_program():
    nc = bacc.Bacc("TRN2", target_bir_lowering=False, debug=False,
                   num_devices=NCORES)
    xt_ap = nc.dram_tensor("xt", [128, NBLK, 128], F32, kind="ExternalInput").ap()
    bd_ap = nc.dram_tensor("bundle", [128, 98], F32, kind="ExternalInput").ap()
    af_ap = nc.dram_tensor("afinal", [64, 1, 32], F32, kind="ExternalOutput").ap()
    sm_ap = nc.dram_tensor("sums", [2, 1056], F32, kind="ExternalOutput").ap()

    with tile.TileContext(nc) as tc, ExitStack() as ctx:
        pc = ctx.enter_context(tc.tile_pool(name="const", bufs=1))
        px = ctx.enter_context(tc.tile_pool(name="x", bufs=1))
        pst = [ctx.enter_context(tc.tile_pool(name=f"st{s}", bufs=2))
               for s in range(SG)]
        # one PSUM bank per supergroup suffices: MM_{k+1} already waits on
        # mult_k (its rhs), which is also the WAR dependency for the bank
        pps = [ctx.enter_context(tc.tile_pool(name=f"ps{s}", bufs=1, space="PSUM"))
               for s in range(SG)]
        psm = ctx.enter_context(tc.tile_pool(name="psums", bufs=1, space="PSUM"))

        # ---- preamble ----
        bundle = pc.tile([128, 98], F32)
        nc.sync.dma_start(bundle[:], bd_ap)
        # bundle cols: 0:64 transition (stacked twice vertically), 64:96
        # onehot reset (rows 0:64), 96 ones, 97 -d bias

        # walrus in this pipeline encodes at most ONE semaphore wait per
        # compute/NoOp instruction, so the whole program is structured to a
        # strict 1-wait discipline: the only pre-barrier op is the bundle
        # DMA (barrier NOP waits on its queue sem alone), every post-barrier
        # instruction joins at most one foreign engine stream, and joins of
        # two streams go through same-engine observer ops.
        tc.strict_bb_all_engine_barrier()

        states = []
        for s in range(SG):
            t0 = pst[s].tile([128, SGW, 32], F32, tag="st")
            nc.vector.memset(t0[:], 1.0)
            states.append(t0)
        collectA = pc.tile([1, 1056], F32)
        nc.vector.memset(collectA[:], 0.0)
        scratch = pc.tile([1, 4], F32)
        nc.vector.memset(scratch[:], 0.0)

        # E = exp(transition); reads bundle (covered by the barrier => no wait)
        E = pc.tile([128, 64], F32)
        nc.scalar.activation(E[:], bundle[:, 0:64], EXP, bias=0.0, scale=1.0)
        collectB = pc.tile([1, 1056], F32)
        nc.scalar.activation(collectB[:], collectA[:], mybir.ActivationFunctionType.Copy)

        # ---- X streaming: 4 big window DMAs (SP ring, FIFO order) ----
        xraw = px.tile([128, NBLK, 128], F32)
        ex = px.tile([128, NBLK, 128], F32)
        for w in range(4):
            b0, b1 = WIN_BASE_BLK[w], WIN_BASE_BLK[w] + WIN_BLKS[w]
            nc.sync.dma_start(xraw[:, b0:b1, :], xt_ap[:, b0:b1, :])
        bias_ap = bundle[:, 97:98]
        exp_last = {}
        for w in range(4):
            for g in range(SG):
                b0 = WIN_BASE_BLK[w] + SGW * g
                b1 = WIN_BASE_BLK[w] + (SGW * (g + 1) if g < SG - 1 else WIN_BLKS[w])
                inst = nc.scalar.activation(ex[:, b0:b1, :], xraw[:, b0:b1, :],
                                            EXP, bias=bias_ap, scale=1.0)
                exp_last[w] = inst

        # ---- main lockstep loop ----
        ones_lo = bundle[0:64, 96:97]
        ones_hi = bundle[64:128, 96:97]
        start_ps = psm.tile([128, 512], F32)
        end_ps = psm.tile([128, 512], F32)
        preT_ps = psm.tile([128, 32], F32)

        # junk matmul: PE observes the DVE memset tick so the first real
        # matmuls carry only the ACT (E) wait
        nc.tensor.matmul(start_ps[0:1, 0:256], lhsT=states[0][0:64, 0, 0:1],
                         rhs=states[0][0:64], start=True, stop=True,
                         tile_position=(0, 0))

        prev22_sg3 = None
        for k in range(SL):
            dummy = None
            if k in (0, 4, 8, 12):
                w = k // 4 if k < K else 2 + (k - K) // 4
                lb = WIN_BASE_BLK[w] + WIN_BLKS[w] - 1
                dummy = nc.vector.tensor_copy(scratch[0:1, 0:1],
                                              ex[0:1, lb, 0:1])
            new_states = []
            for s in range(SG):
                ps = pps[s].tile([128, SGW, 32], F32, tag="ps")
                # float32r: full-rate fp32 streaming for >=256-wide moving ops
                nc.tensor.matmul(ps[0:64], lhsT=E[0:64, :].bitcast(F32R),
                                 rhs=states[s][0:64].bitcast(F32R),
                                 start=True, stop=True, tile_position=(0, 0))
                nc.tensor.matmul(ps[64:128], lhsT=E[64:128, :].bitcast(F32R),
                                 rhs=states[s][64:128].bitcast(F32R),
                                 start=True, stop=True, tile_position=(64, 64))
                bix, u = _mult_src(k, s)
                nst = pst[s].tile([128, SGW, 32], F32, tag="st")
                m = nc.vector.tensor_mul(nst[:], ps[:],
                                         ex[:, bix:bix + SGW, u:u + 32])
                if dummy is not None:
                    add_dep_helper(m.ins, dummy.ins, sync=False,
                                   reason="observe new exp piece first")
                new_states.append(nst)
            states = new_states

            if k == K - 1:
                # reset chunk 0 (sg 0, rows 0:64, chunk col 0) to the exact
                # one-hot init, then record per-chunk start sums
                nc.vector.tensor_copy(states[0][0:64, 0, :],
                                      bundle[0:64, 64:96])
                for s in range(SG):
                    nc.tensor.matmul(start_ps[0:1, s * 256:(s + 1) * 256],
                                     lhsT=ones_lo, rhs=states[s][0:64],
                                     start=True, stop=True,
                                     tile_position=(0, 0))
                    nc.tensor.matmul(start_ps[64:65, s * 256:(s + 1) * 256],
                                     lhsT=ones_hi, rhs=states[s][64:128],
                                     start=True, stop=True,
                                     tile_position=(64, 64))
                nc.vector.tensor_copy(collectA[0:1, 0:512], start_ps[0:1, :])
                nc.scalar.copy(collectB[0:1, 0:512], start_ps[64:65, :])
            if k == SL - 2:
                prev22_sg3 = states[SG - 1]

        # preT: chunk 31 state before its last step
        nc.tensor.matmul(preT_ps[64:65, :], lhsT=ones_hi,
                         rhs=prev22_sg3[64:128, 7, :],
                         start=True, stop=True, tile_position=(64, 64))
        for s in range(SG):
            nc.tensor.matmul(end_ps[0:1, s * 256:(s + 1) * 256],
                             lhsT=ones_lo, rhs=states[s][0:64],
                             start=True, stop=True, tile_position=(0, 0))
            nc.tensor.matmul(end_ps[64:65, s * 256:(s + 1) * 256],
                             lhsT=ones_hi, rhs=states[s][64:128],
                             start=True, stop=True, tile_position=(64, 64))
        nc.vector.tensor_copy(collectA[0:1, 512:1024], end_ps[0:1, :])
        nc.scalar.copy(collectB[0:1, 512:1024], end_ps[64:65, :])
        nc.scalar.copy(collectB[0:1, 1024:1056], preT_ps[64:65, :])

        nc.sync.dma_start(af_ap, states[SG - 1][64:128, 7:8, :])
        nc.sync.dma_start(sm_ap[0:1, :], collectA[:])
        nc.sync.dma_start(sm_ap[1:2, :], collectB[:])
    nc.compile()
    return nc


_prog_cache = {}


def _get_program():
    if "nc" not in _prog_cache:
        _prog_cache["nc"] = _build_program()
    return _prog_cache["nc"]


def _compute_d(X, transition):
    """Mean per-step log growth of the total exp-space mass, from a short
    host-side probe.  Any value within ~+-0.1 keeps A in f32 range."""
    E = np.exp(transition.astype(np.float64))
    a = np.zeros((16, L), np.float64)
    a[:, 0] = 1.0
    tot, n = 0.0, 0
    for t in range(96):
        a = np.exp(X[:16, t, :].astype(np.float64)) * (a @ E)
        sm = a.sum()
        a /= sm
        if t >= 4:
            tot += np.log(sm)
            n += 1
    return float(np.clip(tot / n, 4.5, 5.9))


def _pack_core(Xc, d):
    """Xc [32, T, L] -> window-major device layout [128, NBLK, 128] f32."""
    Y = np.ascontiguousarray(Xc.transpose(2, 1, 0)).astype(np.float32)  # [L, T, 32]
    out = np.zeros((128, NBLK, 128), np.float32)
    # windows 0,1: upper-half stripes k in [4w, 4w+4) of blocks j=0..16
    for w in (0, 1):
        for j in range(17):
            for kk in range(4):
                k = 4 * w + kk
                t_loc = 16 * j - 8 + k
                dst = out[:, WIN_BASE_BLK[w] + j, 32 * kk:32 * kk + 32]
                if t_loc >= 0:
                    dst[0:64] = Y[:, t_loc, :]
                dst[64:128] = Y[:, 256 + t_loc, :]
    # windows 2,3: lower-half stripes m in [4(w-2), 4(w-2)+4) of blocks 1..16
    for w in (2, 3):
        for jj in range(16):
            for kk in range(4):
                m = 4 * (w - 2) + kk
                t_loc = 16 * jj + m
                dst = out[:, WIN_BASE_BLK[w] + jj, 32 * kk:32 * kk + 32]
                dst[0:64] = Y[:, t_loc, :]
                dst[64:128] = Y[:, 256 + t_loc, :]
    return out


def _make_bundle(transition, d):
    bd = np.zeros((128, 98), np.float32)
    tr = transition.astype(np.float32)
    bd[0:64, 0:64] = tr
    bd[64:128, 0:64] = tr
    bd[0, 64:96] = 1.0            # one-hot rows: row B_IDX=0 of the reset block
    bd[:, 96] = 1.0               # ones column
    bd[:, 97] = -d                # exp bias
    return bd


def kernel(X, transition):
    X = np.asarray(X, dtype=np.float32)
    transition = np.asarray(transition, dtype=np.float32)
    d = _compute_d(X, transition)

    bundle = _make_bundle(transition, d)
    in_maps = []
    for c in range(NCORES):
        xt = _pack_core(X[c * BC:(c + 1) * BC], d)
        in_maps.append({"xt": xt, "bundle": bundle})

    nc = _get_program()
    res = run_bass_kernel_spmd(nc, in_maps, core_ids=list(range(NCORES)))

    alpha = np.empty((B, L), np.float64)
    dS = float(d) * S
    with np.errstate(divide="ignore"):
        for c in range(NCORES):
            r = res.results[c]
            sums = r["sums"].astype(np.float64)
            af = r["afinal"].reshape(64, 32).astype(np.float64)
            start = np.empty((C, BC))
            end = np.empty((C, BC))
            start[:16] = sums[0, 0:512].reshape(16, BC)
            start[16:] = sums[1, 0:512].reshape(16, BC)
            end[:16] = sums[0, 512:1024].reshape(16, BC)
            end[16:] = sums[1, 512:1024].reshape(16, BC)
            preT = sums[1, 1024:1056]
            lam = np.zeros(BC)
            for cc in range(C - 1):
                lam += dS + np.log(end[cc]) - np.log(start[cc])
            base = lam - np.log(start[C - 1])
            blk = alpha[c * BC:(c + 1) * BC]
            blk[:] = (base[:, None] + dS + np.log(af).T)
            lse_preT = base + (dS - d) + np.log(preT)
            blk[:, 0] = NEG + lse_preT + X[c * BC:(c + 1) * BC, T - 1, 0].astype(np.float64)
    return alpha.astype(np.float32)


# revision 12
# speedup vs baseline: 1.7060x; 1.1516x over previous
"""CRF forward (alpha) recursion on 8 Trainium2 NeuronCores.

Strategy
--------
Data-parallel over batch: each core gets 32 of 256 batch rows.

Per core, the T=512 recurrence is run in *exp space*:
    A_{t+1}[nxt, b] = exp(x_{t+1}[nxt, b] - d) * sum_prev E[prev, nxt] * A_t[prev, b]
with E = exp(transition) loaded once as the PE stationary operand, and a
constant per-step normalizer d (the mean log-growth rate) keeping A in f32
range.  Each step is then exactly one small matmul (PE) + one elementwise
multiply (DVE).

The serial chain over T is broken via chunked speculation: T is split into
C=32 chunks of S=16 steps.  Every chunk runs K=8 warm-up steps on the
preceding chunk's data starting from a uniform state; the direction of the
CRF forward state forgets its initial condition at a measured rate of
<1e-7 in ~10 steps, so after warm-up each chunk's state equals the true
state up to a per-batch scalar.  The scalars are recovered exactly from
per-chunk column sums (ones-matmuls) and telescoped on the host in f64.
All 32 chunks advance in lockstep (24 slots instead of 512 serial steps),
packed 8 chunks per [128, 4*32] state tile (two 64x64 PE quadrants).

The masked transition column (into 'B', exactly -10000 => exp == 0 in f32)
makes alpha[:, 0] equal -inf in exp space; it is reconstructed exactly as
-10000 + lse(alpha_{T-1}) + x_{T-1, 0} from an extra ones-matmul.
"""

import numpy as np
from contextlib import ExitStack

import concourse.bass as bass
import concourse.bacc as bacc
import concourse.tile as tile
from concourse import mybir
from concourse.bass_utils import run_bass_kernel_spmd
from concourse.tile_rust import add_dep_helper

F32 = mybir.dt.float32
F32R = mybir.dt.float32r
EXP = mybir.ActivationFunctionType.Exp

NCORES = 8
B, T, L = 256, 512, 64
BC = B // NCORES          # batch per core = 32
C = 32                    # chunks
S = T // C                # steps per chunk = 16
K = 8                     # warm-up steps
SL = K + S                # lockstep slots = 24
SG = 2                    # supergroups (16 chunks each: 8 in rows 0:64, 8 in rows 64:128)
SGW = 8                   # chunks per supergroup row-half
NEG = -10000.0

# window-major X layout: windows 0,1 hold the "upper half" stripes (warm-up
# cells) of blocks 0..16, windows 2,3 the "lower half" stripes of blocks
# 1..16.  Widths in 128-col blocks:
WIN_BLKS = [17, 17, 16, 16]
WIN_BASE_BLK = [0, 17, 34, 50]       # cumulative, in blocks
NBLK = 66                            # total blocks
NCOL = NBLK * 128                    # 8448 f32 columns


def _mult_src(k, s):
    """(block_index_start, inner_col) of the Ex slice read by supergroup s
    at slot k: AP [128][SGW blocks][32]."""
    if k < K:
        w = k // 4
        return WIN_BASE_BLK[w] + SGW * s, 32 * (k % 4)
    elif k < K + 8:
        w = 2 + (k - K) // 4
        return WIN_BASE_BLK[w] + SGW * s, 32 * ((k - K) % 4)
    else:
        w = (k - 16) // 4
        return WIN_BASE_BLK[w] + SGW * s + 1, 32 * ((k - 16) % 4)


def _build_program():
    nc = bacc.Bacc("TRN2", target_bir_lowering=False, debug=False,
                   num_devices=NCORES)
    xt_ap = nc.dram_tensor("xt", [128, NBLK, 128], F32, kind="ExternalInput").ap()
    bd_ap = nc.dram_tensor("bundle", [128, 164], F32, kind="ExternalInput").ap()
    af_ap = nc.dram_tensor("afinal", [64, 1, 32], F32, kind="ExternalOutput").ap()
    sm_ap = nc.dram_tensor("sums", [2, 1056], F32, kind="ExternalOutput").ap()

    with tile.TileContext(nc) as tc, ExitStack() as ctx:
        pc = ctx.enter_context(tc.tile_pool(name="const", bufs=1))
        px = ctx.enter_context(tc.tile_pool(name="x", bufs=1))
        pst = [ctx.enter_context(tc.tile_pool(name=f"st{s}", bufs=2))
               for s in range(SG)]
        # one PSUM bank per supergroup suffices: MM_{k+1} already waits on
        # mult_k (its rhs), which is also the WAR dependency for the bank
        pps = [ctx.enter_context(tc.tile_pool(name=f"ps{s}", bufs=1, space="PSUM"))
               for s in range(SG)]
        psm = ctx.enter_context(tc.tile_pool(name="psums", bufs=1, space="PSUM"))

        # ---- preamble ----
        bundle = pc.tile([128, 164], F32)
        nc.sync.dma_start(bundle[:], bd_ap)
        # bundle cols: 0:128 block-diagonal transition (off-diagonal -1e4 so
        # exp gives exact zeros), 128:160 onehot reset (rows 0:64),
        # 160:162 half-ones pair, 162 -d bias

        # walrus in this pipeline encodes at most ONE semaphore wait per
        # compute/NoOp instruction, so the whole program is structured to a
        # strict 1-wait discipline: the only pre-barrier op is the bundle
        # DMA (barrier NOP waits on its queue sem alone), every post-barrier
        # instruction joins at most one foreign engine stream, and joins of
        # two streams go through same-engine observer ops.
        tc.strict_bb_all_engine_barrier()

        states = []
        for s in range(SG):
            t0 = pst[s].tile([128, SGW, 32], F32, tag="st")
            nc.vector.memset(t0[:], 1.0)
            states.append(t0)
        collect = pc.tile([2, 1056], F32)
        nc.vector.memset(collect[:], 0.0)
        scratch = pc.tile([1, 4], F32)
        nc.vector.memset(scratch[:], 0.0)

        # E = exp(block-diag transition); covered by the barrier => no wait.
        # f32r-rounded so the full-K=128 f32r matmuls can consume it.
        E = pc.tile([128, 128], F32)
        nc.scalar.activation(E[:].bitcast(F32R), bundle[:, 0:128], EXP,
                             bias=0.0, scale=1.0)

        # ---- X streaming: 4 big window DMAs (SP ring, FIFO order) ----
        xraw = px.tile([128, NBLK, 128], F32)
        ex = px.tile([128, NBLK, 128], F32)
        for w in range(4):
            b0, b1 = WIN_BASE_BLK[w], WIN_BASE_BLK[w] + WIN_BLKS[w]
            nc.sync.dma_start(xraw[:, b0:b1, :], xt_ap[:, b0:b1, :])
        bias_ap = bundle[:, 162:163]
        exp_last = {}
        for w in range(4):
            for g in range(SG):
                b0 = WIN_BASE_BLK[w] + SGW * g
                b1 = WIN_BASE_BLK[w] + (SGW * (g + 1) if g < SG - 1 else WIN_BLKS[w])
                inst = nc.scalar.activation(ex[:, b0:b1, :], xraw[:, b0:b1, :],
                                            EXP, bias=bias_ap, scale=1.0)
                exp_last[w] = inst

        # ---- main lockstep loop ----
        ones2 = bundle[:, 160:162]        # [128, 2]: col 0 sums rows 0:64, col 1 rows 64:128
        start_ps = psm.tile([2, 512], F32)
        end_ps = psm.tile([2, 512], F32)
        preT_ps = psm.tile([2, 32], F32)

        # junk matmul: PE observes the DVE memset tick so the first real
        # matmuls carry only the ACT (E) wait
        nc.tensor.matmul(start_ps[0:1, 0:256], lhsT=states[0][0:64, 0, 0:1],
                         rhs=states[0][0:64], start=True, stop=True)

        prev22_sg3 = None
        for k in range(SL):
            dummy = None
            if k in (0, 4, 8, 12):
                w = k // 4 if k < K else 2 + (k - K) // 4
                lb = WIN_BASE_BLK[w] + WIN_BLKS[w] - 1
                dummy = nc.vector.tensor_copy(scratch[0:1, 0:1],
                                              ex[0:1, lb, 0:1])
            new_states = []
            for s in range(SG):
                ps = pps[s].tile([128, SGW, 32], F32, tag="ps")
                # one full-K=128 f32r matmul; the block-diagonal stationary
                # keeps the two row-halves independent, and f32r streams
                # fp32 data at full rate for the 256-wide moving operand
                nc.tensor.matmul(ps[:], lhsT=E[:].bitcast(F32R),
                                 rhs=states[s][:].bitcast(F32R),
                                 start=True, stop=True)
                bix, u = _mult_src(k, s)
                nst = pst[s].tile([128, SGW, 32], F32, tag="st")
                m = nc.vector.tensor_mul(nst[:].bitcast(F32R), ps[:],
                                         ex[:, bix:bix + SGW, u:u + 32])
                if dummy is not None:
                    add_dep_helper(m.ins, dummy.ins, sync=False,
                                   reason="observe new exp piece first")
                new_states.append(nst)
            states = new_states

            if k == K - 1:
                # reset chunk 0 (sg 0, rows 0:64, chunk col 0) to the exact
                # one-hot init, then record per-chunk start sums
                nc.vector.tensor_copy(states[0][0:64, 0, :].bitcast(F32R),
                                      bundle[0:64, 128:160])
                for s in range(SG):
                    nc.tensor.matmul(start_ps[0:2, s * 256:(s + 1) * 256],
                                     lhsT=ones2, rhs=states[s][:],
                                     start=True, stop=True)
                nc.vector.tensor_copy(collect[0:2, 0:512], start_ps[0:2, :])
            if k == SL - 2:
                prev22_sg3 = states[SG - 1]

        # preT: chunk 31 state before its last step
        nc.tensor.matmul(preT_ps[0:2, :], lhsT=ones2,
                         rhs=prev22_sg3[:, 7, :],
                         start=True, stop=True)
        for s in range(SG):
            nc.tensor.matmul(end_ps[0:2, s * 256:(s + 1) * 256],
                             lhsT=ones2, rhs=states[s][:],
                             start=True, stop=True)
        nc.vector.tensor_copy(collect[0:2, 512:1024], end_ps[0:2, :])
        nc.vector.tensor_copy(collect[0:2, 1024:1056], preT_ps[0:2, :])

        nc.sync.dma_start(af_ap, states[SG - 1][64:128, 7:8, :])
        nc.sync.dma_start(sm_ap, collect[:])
    nc.compile()
    return nc


_prog_cache = {}


def _get_program():
    if "nc" not in _prog_cache:
        _prog_cache["nc"] = _build_program()
    return _prog_cache["nc"]


def _compute_d(X, transition):
    """Mean per-step log growth of the total exp-space mass, from a short
    host-side probe.  Any value within ~+-0.1 keeps A in f32 range."""
    E = np.exp(transition.astype(np.float64))
    a = np.zeros((16, L), np.float64)
    a[:, 0] = 1.0
    tot, n = 0.0, 0
    for t in range(96):
        a = np.exp(X[:16, t, :].astype(np.float64)) * (a @ E)
        sm = a.sum()
        a /= sm
        if t >= 4:
            tot += np.log(sm)
            n += 1
    return float(np.clip(tot / n, 4.5, 5.9))


def _pack_core(Xc, d):
    """Xc [32, T, L] -> window-major device layout [128, NBLK, 128] f32."""
    Y = np.ascontiguousarray(Xc.transpose(2, 1, 0)).astype(np.float32)  # [L, T, 32]
    out = np.zeros((128, NBLK, 128), np.float32)
    # windows 0,1: upper-half stripes k in [4w, 4w+4) of blocks j=0..16
    for w in (0, 1):
        for j in range(17):
            for kk in range(4):
                k = 4 * w + kk
                t_loc = 16 * j - 8 + k
                dst = out[:, WIN_BASE_BLK[w] + j, 32 * kk:32 * kk + 32]
                if t_loc >= 0:
                    dst[0:64] = Y[:, t_loc, :]
                dst[64:128] = Y[:, 256 + t_loc, :]
    # windows 2,3: lower-half stripes m in [4(w-2), 4(w-2)+4) of blocks 1..16
    for w in (2, 3):
        for jj in range(16):
            for kk in range(4):
                m = 4 * (w - 2) + kk
                t_loc = 16 * jj + m
                dst = out[:, WIN_BASE_BLK[w] + jj, 32 * kk:32 * kk + 32]
                dst[0:64] = Y[:, t_loc, :]
                dst[64:128] = Y[:, 256 + t_loc, :]
    return out


def _make_bundle(transition, d):
    bd = np.zeros((128, 164), np.float32)
    tr = transition.astype(np.float32)
    bd[:, 0:128] = NEG            # off-diagonal blocks -> exp == 0 exactly
    bd[0:64, 0:64] = tr
    bd[64:128, 64:128] = tr
    bd[0, 128:160] = 1.0          # one-hot reset block: row B_IDX=0
    bd[0:64, 160] = 1.0           # half-ones pair for partition sums
    bd[64:128, 161] = 1.0
    bd[:, 162] = -d               # exp bias
    return bd


def kernel(X, transition):
    X = np.asarray(X, dtype=np.float32)
    transition = np.asarray(transition, dtype=np.float32)
    d = _compute_d(X, transition)

    bundle = _make_bundle(transition, d)
    in_maps = []
    for c in range(NCORES):
        xt = _pack_core(X[c * BC:(c + 1) * BC], d)
        in_maps.append({"xt": xt, "bundle": bundle})

    nc = _get_program()
    res = run_bass_kernel_spmd(nc, in_maps, core_ids=list(range(NCORES)))

    alpha = np.empty((B, L), np.float64)
    dS = float(d) * S
    with np.errstate(divide="ignore"):
        for c in range(NCORES):
            r = res.results[c]
            sums = r["sums"].astype(np.float64)
            af = r["afinal"].reshape(64, 32).astype(np.float64)
            start = np.empty((C, BC))
            end = np.empty((C, BC))
            start[:16] = sums[0, 0:512].reshape(16, BC)
            start[16:] = sums[1, 0:512].reshape(16, BC)
            end[:16] = sums[0, 512:1024].reshape(16, BC)
            end[16:] = sums[1, 512:1024].reshape(16, BC)
            preT = sums[1, 1024:1056]
            lam = np.zeros(BC)
            for cc in range(C - 1):
                lam += dS + np.log(end[cc]) - np.log(start[cc])
            base = lam - np.log(start[C - 1])
            blk = alpha[c * BC:(c + 1) * BC]
            blk[:] = (base[:, None] + dS + np.log(af).T)
            lse_preT = base + (dS - d) + np.log(preT)
            blk[:, 0] = NEG + lse_preT + X[c * BC:(c + 1) * BC, T - 1, 0].astype(np.float64)
    return alpha.astype(np.float32)
